# revision 45
# baseline (speedup 1.0000x reference)
"""GQA attention (B=2, S=2048, DIM=4096, H=32, KV=8, HD=128) with interleaved
RoPE + causal mask, distributed over 8 TRN2 NeuronCores.

Sharding: tensor-parallel over KV groups. Core c owns kv-group c (4 query
heads + 1 kv head): it projects Q/K/V for its group over ALL tokens (both
batches), runs causal attention locally, then an AllToAll (split per batch,
overlapped with compute) re-shards the attention output from head-major to
token-major, and each core computes the output projection for 512 tokens
(256 from each batch) against the full wo.

Everything runs in fp16 (PSUM accumulation in fp32): matmuls are 1 cycle/row
on the PE at any width, DVE element-wise ops get 2x/4x fast modes, and DMA
traffic is halved vs fp32. Weights are hoisted into SBUF once (no re-reads);
x / wo stream through in host-packed tile layouts (one DMA per slab).

Phase structure (emission order = pipeline order):
  for tcb 0..7:  P1(tcb): project+RoPE 512 tokens -> qt/kt/v in SBUF
                 P2(b, qc=tcb%4): causal attention for those 512 q-tokens
  AllToAll A emitted after P2(b0) -> overlaps P1/P2 of batch 1
  AllToAll B after P2(b1) -> overlaps P4(A)
  P4: out-projection, batch-0 half then batch-1 half.

Attention bookkeeping per (head, q-chunk of 512): scores via PE (k-major,
column-restricted to the causal region), exp on the Act engine with bias -2
(fp16 overflow margin; cancels in normalization), causal diag handled by a
triu 0/1 multiply on the exp output (no additive mask tables), denominator
via fp16 DVE accumulation + ones-matmul partition reduction, reciprocal via
reciprocal_approx_fast, broadcast back with a K=1 ones-matmul.
"""
import sys
sys.path.insert(0, "/opt/trn_rl_repo")
import numpy as np

B, S, DIM = 2, 2048, 4096
H, KV, HD = 32, 8, 128
SCALE = HD ** -0.5
NC = 8
NT = B * S            # 4096 flat tokens
TC = 512              # token chunk (P1 slab and P2 q-chunk)
ND = DIM // 128       # 32 d-chunks
EXP_BIAS = -2.0       # exp(x-2): keeps fp16 et/dacc far from overflow

_CACHE = {}


def _build():
    import concourse.bacc as bacc
    import concourse.tile as tile
    from concourse import mybir

    F32 = mybir.dt.float32
    F16 = mybir.dt.float16
    EXP = mybir.ActivationFunctionType.Exp

    nc = bacc.Bacc("TRN2", target_bir_lowering=False, num_devices=NC)

    def param(name, shape):
        return nc.declare_dram_parameter(name, shape, F16, isOutput=False)

    # host-packed tile layouts (see _host_prep)
    xh = param("xh", [128, 8 * ND * TC])     # x: [tcb][d][tok]
    wqh = param("wqh", [128, ND * 512])      # my 4 heads' wq: [d][4*128]
    wkh = param("wkh", [128, ND * 128])
    wvh = param("wvh", [128, ND * 128])
    woh = param("woh", [128, 8 * ND * TC])   # wo: [dchunk][fchunk][512]
    cqh = param("cqh", [128, NT])            # RoPE tables, partition-pair
    sqh = param("sqh", [128, NT])            # (cq/sq scaled by 1/sqrt(HD))
    ckh = param("ckh", [128, NT])
    skh = param("skh", [128, NT])
    pswap = param("pswap", [128, 128])       # pair-swap permutation
    ident = param("ident", [128, 128])       # identity (PE transpose)
    ones = param("ones", [128, 128])
    triu = param("triu", [128, 128])         # keep[i,c] = 1 if i <= c
    out_ext = nc.declare_dram_parameter("out", [512, DIM], F16, isOutput=True)

    with tile.TileContext(nc) as tc:
        import contextlib
        with contextlib.ExitStack() as ctx:
            dram = ctx.enter_context(tc.tile_pool(name="dram", bufs=1, space="DRAM"))
            a2a_in = [dram.tile([NC, 512, 256], F16, name=f"a2a_in{i}") for i in range(2)]
            a2a_out = [dram.tile([NC, 512, 256], F16, name=f"a2a_out{i}") for i in range(2)]

            consts = ctx.enter_context(tc.tile_pool(name="consts", bufs=1))
            pswap_sb = consts.tile([128, 128], F16)
            ident_sb = consts.tile([128, 128], F16)
            ones_sb = consts.tile([128, 128], F16)
            triu_sb = consts.tile([128, 128], F16)
            bias_sb = consts.tile([128, 1], F32)
            nc.vector.memset(bias_sb[:, :], EXP_BIAS)
            nc.sync.dma_start(out=pswap_sb, in_=pswap[:, :])
            nc.sync.dma_start(out=ident_sb, in_=ident[:, :])
            nc.sync.dma_start(out=ones_sb, in_=ones[:, :])
            nc.sync.dma_start(out=triu_sb, in_=triu[:, :])

            # phase-1/2 SBUF (released before P4)
            ph12 = contextlib.ExitStack()
            wpool = ph12.enter_context(tc.tile_pool(name="wpool", bufs=1))
            # wq split per head (host layout is head-major) so the first
            # accumulation chain only waits on 1MB of weights
            wq_sb = [wpool.tile([128, ND * 128], F16, tag=f"wq{i}",
                                name=f"wq{i}")
                     for i in range(4)]
            wk_sb = wpool.tile([128, ND * 128], F16, tag="wk")
            wv_sb = wpool.tile([128, ND * 128], F16, tag="wv")
            nc.sync.dma_start(out=wq_sb[0], in_=wqh[:, 0:ND * 128])

            qkv = ph12.enter_context(tc.tile_pool(name="qkv", bufs=1))
            qt_sb = [qkv.tile([128, NT], F16, tag=f"qt{i}", name=f"qt{i}") for i in range(4)]
            kt_sb = qkv.tile([128, NT], F16, tag="kt", name="kt_sb")
            v_sb = qkv.tile([128, NT], F16, tag="vt", name="v_sb")

            xts_p = ph12.enter_context(tc.tile_pool(name="xts", bufs=2))
            tab_p = ph12.enter_context(tc.tile_pool(name="tab", bufs=2))
            rope_p = ph12.enter_context(tc.tile_pool(name="rope", bufs=2))
            acc_p = ph12.enter_context(tc.tile_pool(name="acc", bufs=2, space="PSUM"))
            # phase-2 pools
            sps_p = ph12.enter_context(tc.tile_pool(name="sps", bufs=3, space="PSUM"))
            ops_p = ph12.enter_context(tc.tile_pool(name="ops", bufs=2, space="PSUM"))
            dn_p = ph12.enter_context(tc.tile_pool(name="dn", bufs=1, space="PSUM"))
            et_p = ph12.enter_context(tc.tile_pool(name="et", bufs=3))
            dacc_p = ph12.enter_context(tc.tile_pool(name="dacc", bufs=2))
            osb_p = ph12.enter_context(tc.tile_pool(name="osb", bufs=2))
            nrm_p = ph12.enter_context(tc.tile_pool(name="nrm", bufs=2))

            def emit_p1(tcb, groups=range(6), loads=True):
                ts = slice(tcb * TC, (tcb + 1) * TC)
                # quarter-slabs: first matmul starts after 1MB of x instead of 2
                xh0 = tcb * ND * TC
                QW = ND * TC // 4
                if loads:
                    xts_q = []
                    for q in range(4):
                        xq = xts_p.tile([128, QW], F16, tag=f"xq{q}", name=f"xq{q}")
                        nc.sync.dma_start(out=xq,
                                          in_=xh[:, xh0 + q * QW:xh0 + (q + 1) * QW])
                        xts_q.append(xq)
                    emit_p1.xts_q = xts_q
                else:
                    xts_q = emit_p1.xts_q
                if tcb == 0:
                    # remaining weights, behind tcb0's x slabs in queue order
                    for i in range(1, 4):
                        nc.sync.dma_start(out=wq_sb[i],
                                          in_=wqh[:, i * ND * 128:(i + 1) * ND * 128])
                    nc.sync.dma_start(out=wk_sb, in_=wkh[:, :])
                    nc.sync.dma_start(out=wv_sb, in_=wvh[:, :])
                if loads:
                    cq_t = tab_p.tile([128, TC], F16, tag="cq")
                    sq_t = tab_p.tile([128, TC], F16, tag="sq")
                    ck_t = tab_p.tile([128, TC], F16, tag="ck")
                    sk_t = tab_p.tile([128, TC], F16, tag="sk")
                    nc.sync.dma_start(out=cq_t, in_=cqh[:, ts])
                    nc.sync.dma_start(out=sq_t, in_=sqh[:, ts])
                    nc.sync.dma_start(out=ck_t, in_=ckh[:, ts])
                    nc.sync.dma_start(out=sk_t, in_=skh[:, ts])
                    emit_p1.tabs = (cq_t, sq_t, ck_t, sk_t)
                else:
                    cq_t, sq_t, ck_t, sk_t = emit_p1.tabs

                for g in groups:  # 4 q-heads, k, v
                    acc = acc_p.tile([128, TC], F32, tag="acc")
                    for d in range(ND):
                        if g < 4:
                            lhsT = wq_sb[g][:, d * 128:(d + 1) * 128]
                        elif g == 4:
                            lhsT = wk_sb[:, d * 128:(d + 1) * 128]
                        else:
                            lhsT = wv_sb[:, d * 128:(d + 1) * 128]
                        xts = xts_q[d // 8]
                        nc.tensor.matmul(acc[:, :], lhsT,
                                         xts[:, (d % 8) * TC:(d % 8 + 1) * TC],
                                         start=(d == 0), stop=(d == ND - 1))
                    raw = rope_p.tile([128, TC], F16, tag="raw")
                    nc.vector.tensor_copy(raw[:, :], acc[:, :])
                    if g < 5:
                        # RoPE(t) = t*C + (P@t)*S; swap matmul reuses acc bank
                        nc.tensor.matmul(acc[:, :], pswap_sb[:, :], raw[:, :],
                                         start=True, stop=True)
                        swp = rope_p.tile([128, TC], F16, tag="swp")
                        nc.vector.tensor_copy(swp[:, :], acc[:, :])
                        ct, st_ = (cq_t, sq_t) if g < 4 else (ck_t, sk_t)
                        t1 = rope_p.tile([128, TC], F16, tag="t1")
                        nc.vector.tensor_mul(t1[:, :], raw[:, :], ct[:, :])
                        t2 = rope_p.tile([128, TC], F16, tag="t2")
                        nc.vector.tensor_mul(t2[:, :], swp[:, :], st_[:, :])
                        dst = qt_sb[g][:, ts] if g < 4 else kt_sb[:, ts]
                        nc.vector.tensor_add(dst, t1[:, :], t2[:, :])
                    else:
                        # V: PE-transpose to token-major; reuse acc bank (f16 view)
                        tps = acc.bitcast(F16)[:, 0:TC]
                        for i in range(4):
                            nc.tensor.transpose(tps[:, i * 128:(i + 1) * 128],
                                                raw[:, i * 128:(i + 1) * 128],
                                                ident_sb[:, :])
                        nc.vector.tensor_copy(v_sb[:, ts], tps)

            d3_p = ph12.enter_context(tc.tile_pool(name="d3", bufs=1))
            op3_p = ph12.enter_context(tc.tile_pool(name="op3", bufs=1))

            def emit_p2_part1(b, qc, nk1):
                # non-diagonal k-tiles of the terminal chunk (no masking);
                # requires qt for this chunk and kt/v through tile nk1-1
                qs0 = b * S + qc * TC
                st = {}
                for h in range(4):
                    ops = ops_p.tile([128, TC], F32, tag="ops")
                    dacc = d3_p.tile([128, TC], F16, tag=f"d3{h}", name=f"d3{h}")
                    for j in range(nk1):
                        k0 = b * S + j * 128
                        sps = sps_p.tile([128, TC], F32, tag="sps")
                        nc.tensor.matmul(sps[:, :], kt_sb[:, k0:k0 + 128],
                                         qt_sb[h][:, qs0:qs0 + TC],
                                         start=True, stop=True)
                        et = et_p.tile([128, TC], F16, tag="et")
                        nc.scalar.activation(et[:, :], sps[:, :], EXP,
                                             bias=bias_sb[:, :])
                        if j == 0:
                            nc.vector.tensor_copy(dacc[:, :], et[:, :])
                        else:
                            nc.vector.tensor_add(dacc[:, :], dacc[:, :], et[:, :])
                        nc.tensor.matmul(ops[:, :], v_sb[:, k0:k0 + 128], et[:, :],
                                         start=(j == 0), stop=(j == nk1 - 1))
                    opq = op3_p.tile([128, TC], F16, tag=f"op3{h}", name=f"op3{h}")
                    nc.vector.tensor_copy(opq[:, :], ops[:, :])
                    st[h] = (opq, dacc)
                return st

            def emit_p2(b, qc, j_lo=0, part1=None):
                qs0 = b * S + qc * TC
                n_k = 4 * qc + 4

                def emit_norm(h, ops, dacc, opspart=None):
                    dn = dn_p.tile([1, TC], F32, tag="dn")
                    nc.tensor.matmul(dn[0:1, :], ones_sb[:, 0:1], dacc[:, :],
                                     start=True, stop=True)
                    rec32 = nrm_p.tile([1, TC], F32, tag="rec32")
                    with nc.allow_low_precision(reason="approx recip of softmax denom"):
                        nc.vector.reciprocal_approx_fast(rec32[:, :], dn[:, :])
                        rec16 = nrm_p.tile([1, TC], F16, tag="rec16")
                        nc.vector.tensor_copy(rec16[:, :], rec32[:, :])
                    bcp = sps_p.tile([128, TC], F32, tag="sps")
                    nc.tensor.matmul(bcp[:, :], ones_sb[0:1, :], rec16[:, :],
                                     start=True, stop=True)
                    bcp16 = osb_p.tile([128, TC], F16, tag="bcp16")
                    nc.vector.tensor_copy(bcp16[:, :], bcp[:, :])
                    if opspart is not None:
                        cmb = osb_p.tile([128, TC], F16, tag="cmb")
                        nc.vector.tensor_add(cmb[:, :], opspart[:, :], ops[:, :])
                        src = cmb
                    else:
                        src = ops
                    osb = osb_p.tile([128, TC], F16, tag="osb")
                    nc.vector.tensor_mul(osb[:, :], src[:, :], bcp16[:, :])
                    nc.gpsimd.dma_start(
                        out=a2a_in[b][2 * qc, h * 128:(h + 1) * 128, :],
                        in_=osb[:, 0:256])
                    nc.gpsimd.dma_start(
                        out=a2a_in[b][2 * qc + 1, h * 128:(h + 1) * 128, :],
                        in_=osb[:, 256:512])

                pending = None  # deferred normalization args
                for h in range(4):
                    ops = ops_p.tile([128, TC], F32, tag="ops")
                    if part1 is not None:
                        dacc = part1[h][1]
                    else:
                        dacc = dacc_p.tile([128, TC], F16, tag="dacc")
                    for j in range(j_lo, n_k):
                        r = j - 4 * qc
                        w0 = 128 * r if r > 0 else 0
                        k0 = b * S + j * 128
                        sps = sps_p.tile([128, TC], F32, tag="sps")
                        nc.tensor.matmul(sps[:, w0:TC], kt_sb[:, k0:k0 + 128],
                                         qt_sb[h][:, qs0 + w0:qs0 + TC],
                                         start=True, stop=True)
                        et = et_p.tile([128, TC], F16, tag="et")
                        nc.scalar.activation(et[:, w0:TC], sps[:, w0:TC], EXP,
                                             bias=bias_sb[:, :])
                        if r >= 0:
                            if w0 > 0:
                                nc.vector.memset(et[:, 0:w0], 0.0)
                            nc.vector.tensor_mul(et[:, w0:w0 + 128],
                                                 et[:, w0:w0 + 128], triu_sb[:, :])
                        if j == 0:
                            nc.vector.tensor_copy(dacc[:, :], et[:, :])
                        else:
                            nc.vector.tensor_add(dacc[:, :], dacc[:, :], et[:, :])
                        nc.tensor.matmul(ops[:, :], v_sb[:, k0:k0 + 128], et[:, :],
                                         start=(j == j_lo), stop=(j == n_k - 1))
                    if pending is not None:
                        emit_norm(*pending)
                    pending = (h, ops, dacc,
                               part1[h][0] if part1 is not None else None)
                emit_norm(*pending)

            # ---- interleaved P1/P2 emission + split collectives ----
            # terminal chunk (b1,qc3) is split: after tcb7's Q-projection,
            # its non-diagonal k-tiles (j<12, kt/v ready since tcb6) run
            # while K/V(tcb7) projects; only the diagonal band + norms remain
            # after, so the second AllToAll triggers earlier
            for tcb in range(8):
                if tcb == 7:
                    emit_p1(7, groups=range(4))
                    q3p1 = emit_p2_part1(1, 3, 12)
                    emit_p1(7, groups=range(4, 6), loads=False)
                    emit_p2(1, 3, j_lo=12, part1=q3p1)
                else:
                    emit_p1(tcb)
                    emit_p2(tcb // 4, tcb % 4)
                if tcb == 3 or tcb == 7:
                    bb = tcb // 4
                    nc.gpsimd.collective_compute(
                        "AllToAll", mybir.AluOpType.bypass,
                        replica_groups=[list(range(NC))],
                        ins=[a2a_in[bb].opt()], outs=[a2a_out[bb].opt()],
                    )

            ph12.close()  # release P1/P2 SBUF+PSUM before out-proj pools

            # ---------------- P4: output projection (single wo pass) --------
            # Per dchunk: one 4MB wo slab DMA, reused by all 4 token tiles.
            # Emission order A0 A1 B0 A2 B1 ... interleaves batch-A work (only
            # needs AllToAll#A) ahead of batch-B so #B's latency is hidden.
            # ot loads go on the sync queue: A-half loads clear immediately,
            # B-half loads only gate work that waits on AllToAll#B anyway.
            with (
                tc.tile_pool(name="otp", bufs=1) as otp,
                tc.tile_pool(name="wop", bufs=5) as wop,
                tc.tile_pool(name="ysb", bufs=4) as ysbp,
                tc.tile_pool(name="y_ps", bufs=8, space="PSUM") as y_ps,
            ):
                ot_sb = [[otp.tile([128, 256], F16, tag=f"ot{half}_{f}",
                                   name=f"ot{half}_{f}")
                          for f in range(ND)] for half in range(2)]

                def load_ot(half):
                    # half 1 waits on AllToAll#B: keep it off the sync queue so
                    # it cannot head-of-line block later wo-slab DMAs
                    q = nc.sync if half == 0 else nc.gpsimd
                    for f in range(ND):
                        q.dma_start(
                            out=ot_sb[half][f],
                            in_=a2a_out[half][f // 4, (f % 4) * 128:(f % 4 + 1) * 128, :])

                load_ot(0)
                wo_s = {}
                yps = {}
                order = [(0, 0), (1, 0), (2, 0), (3, 0)]
                for k in range(4, 8):
                    order += [(k - 4, 1), (k, 0)]
                order += [(4, 1), (5, 1), (6, 1), (7, 1)]
                first_b = True
                for dc, half in order:
                    if half == 0:
                        wo_s[dc] = wop.tile([128, ND * 512], F16, tag="wo",
                                            name=f"wo{dc}")
                        nc.sync.dma_start(
                            out=wo_s[dc],
                            in_=woh[:, dc * ND * 512:(dc + 1) * ND * 512])
                    elif first_b:
                        load_ot(1)
                        first_b = False
                    yp = [y_ps.tile([128, 512], F32, tag="yps", name=f"yp{t}")
                          for t in range(2)]
                    yps[(dc, half)] = yp
                    for f in range(ND):
                        wo_t = wo_s[dc][:, f * 512:(f + 1) * 512]
                        for tt in range(2):
                            nc.tensor.matmul(yp[tt][:, :],
                                             ot_sb[half][f][:, tt * 128:(tt + 1) * 128],
                                             wo_t,
                                             start=(f == 0), stop=(f == ND - 1))
                    dsl = slice(dc * 512, (dc + 1) * 512)
                    for tt in range(2):
                        t = half * 2 + tt
                        y_sb = ysbp.tile([128, 512], F16, tag="ysb")
                        nc.vector.tensor_copy(y_sb[:, :], yp[tt][:, :])
                        nc.gpsimd.dma_start(
                            out=out_ext[t * 128:(t + 1) * 128, dsl],
                            in_=y_sb[:, :])
    nc.compile()
    return nc


def _host_prep(x, wq, wk, wv, wo, freqs_cos, freqs_sin):
    f16 = np.float16
    xf = np.asarray(x, np.float32).reshape(NT, DIM)
    # xh[p, ((t*ND)+d)*TC + c] = x[t*TC+c, d*128+p]
    xh = np.ascontiguousarray(
        xf.reshape(8, TC, ND, 128).transpose(3, 0, 2, 1).reshape(128, 8 * ND * TC)
    ).astype(f16)

    pos = np.arange(NT) % S

    def cs(scale):
        c = np.empty((128, NT), np.float32)
        s = np.empty((128, NT), np.float32)
        ct, st_ = freqs_cos[pos].T * scale, freqs_sin[pos].T
        c[0::2] = ct
        c[1::2] = ct
        s[0::2] = -st_ * scale
        s[1::2] = st_ * scale
        return c.astype(f16), s.astype(f16)

    cq_, sq_ = cs(np.float32(SCALE))
    ck_, sk_ = cs(np.float32(1.0))

    pswap = np.zeros((128, 128), f16)
    for i in range(128):
        pswap[i, i ^ 1] = 1.0
    ident = np.eye(128, dtype=f16)
    ones = np.ones((128, 128), f16)
    triu = np.triu(np.ones((128, 128), np.float32)).astype(f16)

    woh = np.ascontiguousarray(
        np.asarray(wo, np.float32).reshape(ND, 128, 8, 512)
        .transpose(1, 2, 0, 3).reshape(128, 8 * ND * 512)
    ).astype(f16)

    per_core = []
    for c in range(NC):
        wqc = np.asarray(wq[:, c * 512:(c + 1) * 512], np.float32)
        wkc = np.asarray(wk[:, c * 128:(c + 1) * 128], np.float32)
        wvc = np.asarray(wv[:, c * 128:(c + 1) * 128], np.float32)
        # head-major: wqh[p, (g*ND + d)*128 + cc] = wq[d*128+p, c*512 + g*128+cc]
        wqh = np.ascontiguousarray(
            wqc.reshape(ND, 128, 4, 128).transpose(1, 2, 0, 3).reshape(128, ND * 512)
        ).astype(f16)
        wkh = np.ascontiguousarray(
            wkc.reshape(ND, 128, 128).transpose(1, 0, 2).reshape(128, ND * 128)
        ).astype(f16)
        wvh = np.ascontiguousarray(
            wvc.reshape(ND, 128, 128).transpose(1, 0, 2).reshape(128, ND * 128)
        ).astype(f16)
        per_core.append({
            "xh": xh, "wqh": wqh, "wkh": wkh, "wvh": wvh, "woh": woh,
            "cqh": cq_, "sqh": sq_, "ckh": ck_, "skh": sk_,
            "pswap": pswap, "ident": ident, "ones": ones, "triu": triu,
        })
    return per_core


def _make_in_maps(inputs):
    return _host_prep(inputs["x"], inputs["wq"], inputs["wk"], inputs["wv"],
                      inputs["wo"], np.asarray(inputs["freqs_cos"], np.float32),
                      np.asarray(inputs["freqs_sin"], np.float32))


def kernel(x, wq, wk, wv, wo, freqs_cos, freqs_sin, mask, positions):
    from concourse.bass_utils import run_bass_kernel_spmd

    if "nc" not in _CACHE:
        _CACHE["nc"] = _build()
    nc = _CACHE["nc"]

    in_maps = _make_in_maps({
        "x": x, "wq": wq, "wk": wk, "wv": wv, "wo": wo,
        "freqs_cos": freqs_cos, "freqs_sin": freqs_sin,
    })
    res = run_bass_kernel_spmd(nc, in_maps, core_ids=list(range(NC)))
    out = np.empty((NT, DIM), np.float32)
    for c in range(NC):
        y = np.asarray(res.results[c]["out"], np.float32)
        out[256 * c:256 * (c + 1), :] = y[0:256]
        out[S + 256 * c:S + 256 * (c + 1), :] = y[256:512]
    return out.reshape(B, S, DIM)
